# revision 20
# baseline (speedup 1.0000x reference)
"""Trainium2 Bass kernel for in-batch contrastive (InfoNCE) loss.

reference math:
    sim = (q @ k.T) / T          # [N, N]
    loss = mean_i( logsumexp_j(sim[i, :]) - sim[i, i] )

Sharding: q rows split across 8 cores (1024 rows each); k replicated.
Each core computes a partial sum of (lse_i - pos_i) over its rows fully
on-device; the host sums the 8 partial scalars and divides by N.

Per-core device pipeline (all static/unrolled, Tile-scheduled):
  for each 128-row chunk m (8 chunks):
    for each column group g of 2048 (4 groups over N=8192):
      PE   : raw dots into PSUM [128, 2048] (f32r matmuls, K=256 in 2 passes)
      DVE  : m_g   = rowmax(psum)                 (raw-dot domain)
      DVE  : c_g   = max(m_g, 0); bias_g = -c_g/T
      ACT  : s_g   = sum_j exp(psum/T + bias_g)   (accum_out, one pass)
    combine groups exactly: c = max_g c_g,
      lse = c/T + ln( sum_g s_g * exp((c_g - c)/T) )
    pos via one fused DVE multiply-reduce on natural-layout q,k rows.
  partial = sum over 1024 rows of (lse - pos)  -> [1,1] -> DRAM
"""

import numpy as np

N = 8192          # rows of q and k
C = 256           # feature dim
TEMP = 0.07
NCORES = 8
RPC = N // NCORES  # 1024 rows per core
P = 128            # partitions
MCH = RPC // P     # 8 row chunks per core
KK = C // P        # 2 contraction chunks
NTILE = 512        # matmul moving free dim
NG = 8             # psum groups per chunk
GC = N // NG       # 2048 cols per group
TPG = GC // NTILE  # 4 matmul col tiles per group


def _build_nc(mm_dtype="f32r"):
    from contextlib import ExitStack

    import concourse.bacc as bacc
    import concourse.tile as tile
    from concourse import bass_isa, mybir

    fp32 = mybir.dt.float32
    bf16 = mybir.dt.bfloat16
    AF = mybir.ActivationFunctionType
    ALU = mybir.AluOpType
    AX = mybir.AxisListType

    nc = bacc.Bacc(
        "TRN2", target_bir_lowering=False, debug=False, num_devices=NCORES
    )

    if mm_dtype == "f32r":
        mmdt = mybir.dt.float32r
    elif mm_dtype == "f32":
        mmdt = fp32
    else:
        raise ValueError(mm_dtype)

    # qT/kT feed the PE only; typed f32r end-to-end (host pre-rounds values
    # to the fp32r grid so the DMA chain is a pure copy).
    qT = nc.dram_tensor("qT", [C, RPC], mmdt, kind="ExternalInput").ap()
    kT = nc.dram_tensor("kT", [C, N], mmdt, kind="ExternalInput").ap()
    ident = nc.dram_tensor("ident", [P, P], fp32, kind="ExternalInput").ap()
    out = nc.dram_tensor("out", [1, 1], fp32, kind="ExternalOutput").ap()

    with tile.TileContext(nc) as tc, ExitStack() as ctx:
        big = ctx.enter_context(tc.tile_pool(name="big", bufs=1))
        stats = ctx.enter_context(tc.tile_pool(name="stats", bufs=1))
        work = ctx.enter_context(tc.tile_pool(name="work", bufs=4))
        escr_pool = ctx.enter_context(tc.tile_pool(name="escr", bufs=2))
        psum = ctx.enter_context(tc.tile_pool(name="psum", bufs=4, space="PSUM"))

        # ---- persistent SBUF inputs ----
        qt_sb = [big.tile([P, RPC], mmdt, name=f"qt{kk}") for kk in range(KK)]
        for kk in range(KK):
            nc.sync.dma_start(out=qt_sb[kk][:], in_=qT[kk * P:(kk + 1) * P, :])

        ident_sb = big.tile([P, P], fp32, name="ident_sb")
        nc.sync.dma_start(out=ident_sb[:], in_=ident[:])

        # k.T column tiles, in the order the PE consumes them
        kt_sb = [[None] * (N // NTILE) for _ in range(KK)]
        for g in range(NG):
            for kk in range(KK):
                for j in range(TPG):
                    t = g * TPG + j
                    kt_sb[kk][t] = big.tile([P, NTILE], mmdt, name=f"kt{kk}_{t}")
                    nc.sync.dma_start(
                        out=kt_sb[kk][t][:],
                        in_=kT[kk * P:(kk + 1) * P, t * NTILE:(t + 1) * NTILE],
                    )

        # ---- persistent stats / accumulators ----
        sg_all = stats.tile([P, MCH, NG], fp32, name="sg_all")
        bias_all = stats.tile([P, MCH, NG], fp32, name="bias_all")
        lse_all = stats.tile([P, MCH], fp32, name="lse_all")
        pos_all = stats.tile([P, MCH], fp32, name="pos_all")
        nsc_all = stats.tile([P, MCH], fp32, name="nsc_all")
        S_all = stats.tile([P, MCH], fp32, name="S_all")
        zero_col = stats.tile([P, 1], fp32, name="zero_col")
        nc.vector.memset(zero_col[:], 0.0)

        inv_t = 1.0 / TEMP

        for m in range(MCH):
            for g in range(NG):
                pg = psum.tile([P, GC], fp32, name="pg")
                for kk in range(KK):
                    lhsT = qt_sb[kk][:, m * P:(m + 1) * P]
                    for j in range(TPG):
                        t = g * TPG + j
                        nc.tensor.matmul(
                            pg[:, j * NTILE:(j + 1) * NTILE],
                            lhsT,
                            kt_sb[kk][t][:],
                            start=(kk == 0),
                            stop=(kk == KK - 1),
                        )

                b_g = bias_all[:, m, g:g + 1]
                # psum already holds x/T (q pre-scaled by 1/T on host);
                # bias = -rowmax(x/T) comes straight out of the reduce
                nc.vector.reduce_max(b_g, pg[:], axis=AX.X, negate=True)
                if g == 0:
                    # pos = diagonal of this chunk's block; kT is rolled per
                    # core so chunk m's diagonal sits at cols m*128..m*128+127
                    dscr = work.tile([P, P], fp32, name="dscr")
                    nc.vector.tensor_tensor(
                        dscr, pg[:, m * P:(m + 1) * P], ident_sb[:], op=ALU.mult
                    )
                    nc.vector.reduce_sum(pos_all[:, m:m + 1], dscr, axis=AX.X)
                # s_g = sum_j exp(x/T - max/T); outputs all in (0, 1]
                esc = escr_pool.tile([P, GC], bf16, name="esc")
                nc.scalar.activation(
                    esc[:],
                    pg[:],
                    AF.Exp,
                    bias=b_g,
                    scale=1.0,
                    accum_out=sg_all[:, m, g:g + 1],
                )

            # ---- combine the NG groups of this chunk exactly ----
            # nsc = min_g bias_g = -c/T  (c = chunk row max of x/T)
            nsc_m = nsc_all[:, m:m + 1]
            nc.vector.tensor_reduce(
                nsc_m, bias_all[:, m, :], axis=AX.X, op=ALU.min
            )
            # ee_g = exp(max_g/T - c/T) = exp(-bias_g + nsc)
            ee = work.tile([P, NG], fp32, name="ee")
            nc.scalar.activation(
                ee[:], bias_all[:, m, :], AF.Exp, bias=nsc_m, scale=-1.0
            )
            # S = sum_g s_g * ee_g   (>= 1); ln deferred to one batched Ln below
            tsc = work.tile([P, NG], fp32, name="tsc")
            nc.vector.tensor_tensor(tsc, sg_all[:, m, :], ee, op=ALU.mult)
            nc.vector.reduce_sum(S_all[:, m:m + 1], tsc, axis=AX.X)

        # ---- per-core partial: sum over all rows of (lse - pos) ----
        # one batched Ln over all chunks (avoids per-chunk ACT table switches)
        lnS_all = stats.tile([P, MCH], fp32, name="lnS_all")
        nc.scalar.activation(
            lnS_all[:], S_all[:], AF.Ln, bias=zero_col[:], scale=1.0
        )
        nc.vector.tensor_tensor(lse_all[:], lnS_all[:], nsc_all[:], op=ALU.subtract)
        lp = stats.tile([P, MCH], fp32, name="lp")
        loss_col = stats.tile([P, 1], fp32, name="loss_col")
        nc.vector.tensor_tensor(lp, lse_all[:], pos_all[:], op=ALU.subtract)
        nc.vector.reduce_sum(loss_col, lp[:], axis=AX.X)
        total_sb = stats.tile([P, 1], fp32, name="total_sb")
        nc.gpsimd.partition_all_reduce(
            total_sb[:], loss_col[:], channels=P, reduce_op=bass_isa.ReduceOp.add
        )
        nc.sync.dma_start(out=out[:], in_=total_sb[0:1, :])

    nc.compile()
    return nc


_NC_CACHE = {}


def _get_nc(mm_dtype="f32r"):
    if mm_dtype not in _NC_CACHE:
        _NC_CACHE[mm_dtype] = _build_nc(mm_dtype)
    return _NC_CACHE[mm_dtype]


def _round_f32r(a):
    """Round fp32 values to the fp32r grid (1s + 8e + 11m in the top 20 bits),
    round-to-nearest-even, low 12 bits zeroed."""
    u = np.ascontiguousarray(a, dtype=np.float32).view(np.uint32)
    r = (u + np.uint32(0x7FF) + ((u >> np.uint32(12)) & np.uint32(1))) & np.uint32(
        0xFFFFF000
    )
    return r.view(np.float32)


def _in_maps(q, k, mm_dtype="f32r"):
    q = np.ascontiguousarray(np.asarray(q, dtype=np.float32))
    k = np.ascontiguousarray(np.asarray(k, dtype=np.float32))
    assert q.shape == (N, C) and k.shape == (N, C)
    rnd = _round_f32r if mm_dtype == "f32r" else (lambda a: a)
    kT = rnd(np.ascontiguousarray(k.T))
    ident = np.eye(P, dtype=np.float32)
    maps = []
    for c in range(NCORES):
        sl = slice(c * RPC, (c + 1) * RPC)
        qc = np.ascontiguousarray(q[sl])
        maps.append(
            {
                "qT": rnd(np.ascontiguousarray(qc.T) * np.float32(1.0 / TEMP)),
                # roll so this core's diagonal block sits at columns 0..RPC-1
                "kT": np.ascontiguousarray(np.roll(kT, -c * RPC, axis=1)),
                "ident": ident,
            }
        )
    return maps


def _run(maps, trace=False, mm_dtype="f32r", **kwargs):
    from concourse.bass_utils import run_bass_kernel_spmd

    nc = _get_nc(mm_dtype)
    return run_bass_kernel_spmd(
        nc, maps, list(range(NCORES)), trace=trace, **kwargs
    )


def kernel(q, k):
    res = _run(_in_maps(q, k))
    total = sum(float(r["out"][0, 0]) for r in res.results)
    return np.float32(total / N)
